# revision 2
# baseline (speedup 1.0000x reference)
"""MeshConv (Chebyshev graph conv, K=6) Trainium2 kernel, 8 NeuronCores.

v2 — optimized for end-to-end wall time. The dominant cost of this problem
in this environment is the ~60 MB/s host<->device tunnel, so the kernel
minimizes transferred bytes and host-side numpy work:

- vertex (dst-row) sharding, identity permutation (no host scatter/gather)
- x shipped as bf16 in natural [B, rows, F] order (41 MB total); the device
  builds the [pos, b*64+f] token layout itself
- edge metadata shipped as int16 gather indices + per-slot (rloc, val)
  pairs (~0.6 MB/core); the [slots x rows] SpMM patterns are built on device
  with an iota-compare (pat[s,r] = (iota[r]==rloc[s]) * val[s])
- per Chebyshev step: AllGather bf16 tokens, dma_gather per-edge source
  tokens, pattern-matmul per 128-row dst tile on the TensorEngine, fused
  DVE recurrence in fp32
- dense projection at the end: xbar-transpose the bf16 token arrays,
  accumulate all K taps in PSUM, write bf16 output in natural layout
"""
import sys

sys.path.insert(0, '/opt/trn_rl_repo')

import numpy as np
import ml_dtypes

import concourse.bass as bass
import concourse.bacc as bacc
import concourse.mybir as mybir
import concourse.tile as tile_mod
from concourse.tile import TileContext
from concourse.bass_utils import run_bass_kernel_spmd

# ---------------------------------------------------------------- constants
B, FIN, K, FOUT = 8, 64, 6, 64
NCORE = 8
TOK = B * FIN              # 512 values per vertex token
M = 40000
SLICE = 5120               # rows per core
T = 40                     # 128-row dst tiles per core
MPAD = NCORE * SLICE       # 40960
NTILE = NCORE * T          # 320
CPT_A, CPT_B = 7, 3        # A/B chunks per tile
CPT = CPT_A + CPT_B
NCH = CPT * T              # pattern chunks per core (400)
NIDX_A = CPT_A * T * 128   # 35840
NIDX_B = CPT_B * T * 128   # 15360
ASPLIT = 32768             # int16 A-zone: src pos < ASPLIT
BBASE = MPAD - 32768       # 8192; B-zone gathers from rows [BBASE, MPAD)
CALL_CH = 8                # gather chunks per dma_gather call (1024 idxs)
XSCALE = 5.5 / 127         # int8 x quantization step (x ~ N(0,1)); 5.5 sigma covers
                           # the sample max (~5.45 over 20.5M), so nothing clips --
                           # clipping inflates max-rel error even though it helps L2

# walrus in this environment accepts only 1 sync-wait per CTRL instruction:
# spread the Tile tail-drain's waits across preceding nops.
def _patched_drain_and_barrier(self, tick_clock, wait_clock):
    nop0 = self.nc.sync.nop(nofuse=True)
    wait_clock.add_sem_waits(nop0.ins, tile_mod.ScopedClock({None: tick_clock.global_clock}))
    si = nop0.ins.sync_info
    waits = list(si.on_wait) if si and si.on_wait else []
    if len(waits) > 1:
        si.on_wait = waits[:1]
        rest = waits[1:]
        while rest:
            n = self.nc.sync.nop(nofuse=True)
            nsi = n.ins.sync_info
            if nsi is None:
                n.ins.sync_info = mybir.SyncInfo(on_wait=rest[:1], on_update=[])
            else:
                nsi.on_wait = rest[:1]
            rest = rest[1:]
    self.nc.sync.drain()
    self.nc.all_engine_barrier()
    assert self.sems is not None
    popped = self.nc._tile_sem_poison_stack.pop()
    assert popped is self._sem_poison
    self.nc.clear_and_free_semaphores(list(self.sems.allocated().values()))
    self.nc.all_engine_barrier()


tile_mod.TileContext._drain_and_barrier = _patched_drain_and_barrier


def _calls(nidx):
    """List of (start_chunk, n_idx) dma_gather calls covering nidx indices."""
    out = []
    ch = 0
    nch_total = nidx // 128
    while ch < nch_total:
        n = min(CALL_CH, nch_total - ch)
        out.append((ch, n * 128))
        ch += n
    return out


# ---------------------------------------------------------------- host prep
def build_meta(edge_rows, edge_cols, edge_vals):
    """Vectorized slotting of the edge list into the fixed chunk grid."""
    er = np.asarray(edge_rows).astype(np.int64)
    ec = np.asarray(edge_cols).astype(np.int64)
    ev = np.asarray(edge_vals).astype(np.float32)

    tile = er >> 7
    cls = np.where(ec < BBASE, 0, np.where(ec < ASPLIT, 1, 2)).astype(np.int8)
    order = np.lexsort((cls, tile))
    t_s, c_s = tile[order], cls[order]
    ec_s, ev_s = ec[order], ev[order]
    rloc_s = (er & 127)[order]

    tstart = np.searchsorted(t_s, np.arange(NTILE + 1))
    ranks = np.arange(er.shape[0]) - tstart[t_s]
    n_t = np.diff(tstart)
    if n_t.max() > CPT * 128:
        raise RuntimeError(f"tile overflow: {n_t.max()} edges > {CPT * 128}")

    capA = CPT_A * 128
    nAB_t = np.bincount(t_s[c_s < 2], minlength=NTILE)
    nA_t = np.minimum(nAB_t, capA)
    asel = (c_s < 2) & (ranks < nA_t[t_s])
    bslot = ranks - nA_t[t_s]
    if not (c_s[~asel] >= 1).all():
        raise RuntimeError("must-A edge spilled to B zone")
    if bslot[~asel].size and bslot[~asel].max() >= CPT_B * 128:
        raise RuntimeError("B zone overflow")

    core = t_s // T
    tl = t_s % T

    idxA = np.zeros((NCORE, NIDX_A), np.int16)
    idxB = np.zeros((NCORE, NIDX_B), np.int16)
    rl = np.zeros((NCORE, 128, NCH), ml_dtypes.bfloat16)
    vv = np.zeros((NCORE, 128, NCH), ml_dtypes.bfloat16)

    a = asel
    sl = ranks[a]
    j, sub = sl >> 7, sl & 127
    idxA[core[a], (tl[a] * CPT_A + j) * 128 + sub] = ec_s[a].astype(np.int16)
    pch = tl[a] * CPT + j
    rl[core[a], sub, pch] = rloc_s[a].astype(ml_dtypes.bfloat16)
    vv[core[a], sub, pch] = ev_s[a].astype(ml_dtypes.bfloat16)

    b = ~asel
    sl = bslot[b]
    j, sub = sl >> 7, sl & 127
    idxB[core[b], (tl[b] * CPT_B + j) * 128 + sub] = (ec_s[b] - BBASE).astype(np.int16)
    pch = tl[b] * CPT + CPT_A + j
    rl[core[b], sub, pch] = rloc_s[b].astype(ml_dtypes.bfloat16)
    vv[core[b], sub, pch] = ev_s[b].astype(ml_dtypes.bfloat16)

    def wrap(idx):
        # dma_gather layout: idx i -> partition i%16, free i//16 (the x8
        # Q7-core replication happens on device)
        n = idx.shape[1]
        return np.ascontiguousarray(idx.reshape(NCORE, n // 16, 16).transpose(0, 2, 1))

    return {"idxA_w": wrap(idxA), "idxB_w": wrap(idxB), "rl": rl, "vv": vv}


# int16-element offsets of the sections inside the per-core metadata blob:
# [wblk bf16 | rloc bf16 | vals bf16 | idxA int16 | idxB int16]
BL_W = 0
BL_RL = BL_W + K * 128 * 128
BL_VV = BL_RL + 128 * NCH
BL_IA = BL_VV + 128 * NCH
BL_IB = BL_IA + NIDX_A
BL_END = BL_IB + NIDX_B


def build_blob(meta, wb):
    wbv = wb.view(np.int16).reshape(-1)
    blob = np.empty((NCORE, 1, BL_END), np.int16)
    for c in range(NCORE):
        blob[c, 0, BL_W:BL_RL] = wbv
        blob[c, 0, BL_RL:BL_VV] = meta["rl"][c].view(np.int16).reshape(-1)
        blob[c, 0, BL_VV:BL_IA] = meta["vv"][c].view(np.int16).reshape(-1)
        blob[c, 0, BL_IA:BL_IB] = meta["idxA_w"][c].reshape(-1)
        blob[c, 0, BL_IB:BL_END] = meta["idxB_w"][c].reshape(-1)
    return blob


def build_w_blocks(W):
    """W [FIN*K, FOUT] -> per-k block-diagonal [128, 128] (2 batches/block)."""
    Wk = np.asarray(W).astype(np.float32).reshape(FIN, K, FOUT)
    blocks = np.zeros((K, 128, 128), np.float32)
    for k in range(K):
        blocks[k, 0:64, 0:64] = Wk[:, k, :]
        blocks[k, 64:128, 64:128] = Wk[:, k, :]
    return blocks.astype(ml_dtypes.bfloat16).reshape(K * 128, 128)


def build_x_slices(x):
    """x [B, M, FIN] fp32 -> per-core int8 [B*SLICE, FIN] (b-major rows)."""
    y = np.asarray(x) * (1.0 / XSCALE)
    np.rint(y, out=y)
    np.clip(y, -127, 127, out=y)
    xq = y.astype(np.int8)  # [B, M, FIN]
    out = np.zeros((NCORE, B * SLICE, FIN), np.int8)
    for c in range(NCORE):
        r0 = c * SLICE
        r1 = min(M, r0 + SLICE)
        out[c].reshape(B, SLICE, FIN)[:, : r1 - r0] = xq[:, r0:r1]
    return out


# ---------------------------------------------------------------- device IR
def build_nc(repeat=1):
    nc = bacc.Bacc(None, target_bir_lowering=False, debug=False,
                   dynamic_dma_scratch_size=16384)
    dt = mybir.dt

    xin = nc.declare_dram_parameter("xin", [B * SLICE, FIN], dt.int8, isOutput=False)
    blob = nc.declare_dram_parameter("blob", [1, BL_END], dt.int16, isOutput=False)
    # outp rows [0, B*SLICE) = int8 output; rows [B*SLICE, +1280) hold the
    # [128, 160] f32 dequant scales, bitcast to int8
    outp = nc.declare_dram_parameter("outp", [B * SLICE + 1280, FOUT], dt.int8, isOutput=True)

    contrib = [nc.dram_tensor(f"contrib{k}", [SLICE, TOK], dt.bfloat16) for k in range(K)]
    gath = [None] + [nc.dram_tensor(f"gath{k}", [MPAD, TOK], dt.bfloat16,
                                    addr_space="Shared") for k in range(1, K)]
    xf = [nc.dram_tensor(f"xf{k}", [SLICE, TOK], dt.float32) for k in range(K)]
    patd = nc.dram_tensor("patd", [NCH * 128, 128], dt.bfloat16)

    a_calls = _calls(NIDX_A)
    b_calls = _calls(NIDX_B)
    ga_free = max(n // 128 for _, n in a_calls)
    gb_free = max(n // 128 for _, n in b_calls)

    with TileContext(nc) as tc:
        with (
            tc.tile_pool(name="io", bufs=1) as io,
            tc.tile_pool(name="st0", bufs=2) as st0,
            tc.tile_pool(name="ga", bufs=2) as gap,
            tc.tile_pool(name="gb", bufs=2) as gbp,
            tc.tile_pool(name="patp", bufs=3) as patp,
            tc.tile_pool(name="ev", bufs=2) as evp,
            tc.tile_pool(name="prj", bufs=7) as prjp,
            tc.tile_pool(name="oc", bufs=2) as ocp,
            tc.tile_pool(name="ps", bufs=3, space="PSUM") as psp,
            tc.tile_pool(name="psj", bufs=4, space="PSUM") as psjp,
        ):
            # ---- resident tiles (packed to dodge 4KB/partition padding) ---
            # idx_t columns: [0, NIDX_A/16) = A indices, then B indices
            IBOFF = NIDX_A // 16
            idx_t = io.tile([128, (NIDX_A + NIDX_B) // 16], dt.int16)
            for r in range(8):
                nc.sync.dma_start(
                    out=idx_t[16 * r:16 * r + 16, :IBOFF],
                    in_=blob[:, BL_IA:BL_IB].rearrange("o (p f) -> p (o f)", p=16))
                nc.sync.dma_start(
                    out=idx_t[16 * r:16 * r + 16, IBOFF:],
                    in_=blob[:, BL_IB:BL_END].rearrange("o (p f) -> p (o f)", p=16))
            w_t = io.tile([128, K, 128], dt.bfloat16)
            nc.sync.dma_start(out=w_t[:], in_=blob[:, BL_W:BL_RL].bitcast(
                dt.bfloat16).rearrange("o (k p r) -> p k (o r)", p=128, k=K))
            iota32 = io.tile([128, 128], dt.int32)
            nc.gpsimd.iota(iota32[:], [[1, 128]], channel_multiplier=0)
            iota_t = io.tile([128, 128], dt.float32)
            nc.vector.tensor_copy(iota_t[:], iota32[:])
            stg_t = io.tile([128, 2 * NCH], dt.bfloat16)
            nc.sync.dma_start(out=stg_t[:, :NCH], in_=blob[:, BL_RL:BL_VV].bitcast(
                dt.bfloat16).rearrange("o (p f) -> p (o f)", p=128))
            nc.sync.dma_start(out=stg_t[:, NCH:], in_=blob[:, BL_VV:BL_IA].bitcast(
                dt.bfloat16).rearrange("o (p f) -> p (o f)", p=128))
            # f32c columns: [0,NCH) = rloc, [NCH,2NCH) = vals, [2NCH] = 1.0
            f32c = io.tile([128, 2 * NCH + 1], dt.float32)
            nc.vector.tensor_copy(f32c[:, :NCH], stg_t[:, :NCH])
            nc.vector.tensor_copy(f32c[:, NCH:2 * NCH], stg_t[:, NCH:])
            nc.vector.memset(f32c[:, 2 * NCH:2 * NCH + 1], 1.0)
            amax_all = io.tile([128, 4 * T], dt.float32)

            # ---- pattern build: pat[s, r] = (iota[r] == rl[s]) * vv[s] ----
            def build_patterns():
                for c0 in range(0, NCH, CPT):
                    pt = patp.tile([128, CPT, 128], dt.bfloat16, tag="patbuild")
                    for j in range(CPT):
                        nc.vector.tensor_scalar(
                            pt[:, j, :], iota_t[:],
                            f32c[:, c0 + j:c0 + j + 1],
                            f32c[:, NCH + c0 + j:NCH + c0 + j + 1],
                            op0=mybir.AluOpType.is_equal, op1=mybir.AluOpType.mult)
                    nc.sync.dma_start(
                        out=patd[:].rearrange("(c s) r -> s c r", s=128)[:, c0:c0 + CPT, :],
                        in_=pt[:])

            # ---- stage0: xin int8 [B*SLICE, F] -> token layout ------------
            def stage0():
                GS = 2
                for g in range(0, T, GS):
                    t0i = st0.tile([128, GS, TOK], dt.int8, tag="s0i")
                    for b in range(B):
                        nc.sync.dma_start(
                            out=t0i[:, :, b * FIN:(b + 1) * FIN],
                            in_=xin[b * SLICE:(b + 1) * SLICE, :].rearrange(
                                "(t p) f -> p t f", p=128)[:, g:g + GS, :])
                    t0c = st0.tile([128, GS, TOK], dt.float32, tag="s0c")
                    nc.vector.tensor_copy(t0c[:], t0i[:])
                    t0f = st0.tile([128, GS, TOK], dt.float32, tag="s0f")
                    nc.vector.tensor_scalar(t0f[:], t0c[:], float(XSCALE), None,
                                            op0=mybir.AluOpType.mult)
                    t0b = st0.tile([128, GS, TOK], dt.bfloat16, tag="s0b")
                    nc.vector.tensor_copy(t0b[:], t0f[:])
                    nc.sync.dma_start(
                        out=contrib[0][:].rearrange("(a p) f -> p a f", p=128)[:, g:g + GS, :],
                        in_=t0b[:])
                    nc.sync.dma_start(
                        out=xf[0][:].rearrange("(a p) f -> p a f", p=128)[:, g:g + GS, :],
                        in_=t0f[:])

            # ---- one Chebyshev step ---------------------------------------
            def cheb_step(k):
                gk = gath[k]
                nc.gpsimd.collective_compute(
                    "AllGather", mybir.AluOpType.bypass,
                    replica_groups=[list(range(NCORE))],
                    ins=[contrib[k - 1][:]], outs=[gk[:]],
                )
                GA, GB = [], []
                for (ch0, n) in a_calls:
                    g = gap.tile([128, ga_free, TOK], dt.bfloat16, tag="ga")
                    nc.gpsimd.dma_gather(
                        out_ap=g[:, : n // 128, :], in_ap=gk[0:ASPLIT, :],
                        idxs_ap=idx_t[:, ch0 * 8: ch0 * 8 + n // 16],
                        num_idxs=n, num_idxs_reg=n, elem_size=TOK,
                        single_packet=False)
                    GA.append((ch0, g))
                for (ch0, n) in b_calls:
                    g = gbp.tile([128, gb_free, TOK], dt.bfloat16, tag="gb")
                    nc.gpsimd.dma_gather(
                        out_ap=g[:, : n // 128, :], in_ap=gk[BBASE:, :],
                        idxs_ap=idx_t[:, IBOFF + ch0 * 8: IBOFF + ch0 * 8 + n // 16],
                        num_idxs=n, num_idxs_reg=n, elem_size=TOK,
                        single_packet=False)
                    GB.append((ch0, g))

                def slot(lists, ch):
                    for ch0, g in lists:
                        if ch0 <= ch < ch0 + CALL_CH:
                            return g[:, ch - ch0, :]
                    raise AssertionError

                for tl in range(T):
                    pt = patp.tile([128, CPT, 128], dt.bfloat16, tag="pat")
                    nc.sync.dma_start(out=pt[:], in_=patd[:].rearrange(
                        "(c s) r -> s c r", s=128)[:, tl * CPT:(tl + 1) * CPT, :])
                    ps = psp.tile([128, TOK], dt.float32, tag="ps")
                    for j in range(CPT_A):
                        nc.tensor.matmul(ps[:], pt[:, j, :], slot(GA, tl * CPT_A + j),
                                         start=(j == 0), stop=False)
                    for j in range(CPT_B):
                        nc.tensor.matmul(ps[:], pt[:, CPT_A + j, :], slot(GB, tl * CPT_B + j),
                                         start=False, stop=(j == CPT_B - 1))
                    # recurrence: k=1: x1 = ps - x0 ; k>1: xk = 2 ps - 2 x_{k-1} - x_{k-2}
                    xprev = evp.tile([128, TOK], dt.float32, tag="xprev")
                    nc.sync.dma_start(out=xprev[:], in_=xf[k - 1][tl * 128:(tl + 1) * 128, :])
                    xk_t = evp.tile([128, TOK], dt.float32, tag="xk")
                    if k == 1:
                        nc.vector.scalar_tensor_tensor(
                            xk_t[:], ps[:], 1.0, xprev[:],
                            op0=mybir.AluOpType.mult, op1=mybir.AluOpType.subtract)
                    else:
                        xpp = evp.tile([128, TOK], dt.float32, tag="xpp")
                        nc.sync.dma_start(out=xpp[:], in_=xf[k - 2][tl * 128:(tl + 1) * 128, :])
                        tmp = evp.tile([128, TOK], dt.float32, tag="tmp")
                        nc.vector.scalar_tensor_tensor(
                            tmp[:], xprev[:], 2.0, xpp[:],
                            op0=mybir.AluOpType.mult, op1=mybir.AluOpType.add)
                        nc.vector.scalar_tensor_tensor(
                            xk_t[:], ps[:], 2.0, tmp[:],
                            op0=mybir.AluOpType.mult, op1=mybir.AluOpType.subtract)
                    if k < K - 1:
                        nc.sync.dma_start(out=xf[k][tl * 128:(tl + 1) * 128, :], in_=xk_t[:])
                    xkb = evp.tile([128, TOK], dt.bfloat16, tag="xkb")
                    nc.vector.tensor_copy(xkb[:], xk_t[:])
                    nc.sync.dma_start(out=contrib[k][tl * 128:(tl + 1) * 128, :], in_=xkb[:])

            # ---- dense projection (all K taps, PSUM-accumulated) ----------
            def projection():
                HS = SLICE // 2
                for j in range(4):
                    for h in range(2):
                        xT = []
                        for k in range(K):
                            t = prjp.tile([128, HS], dt.bfloat16, tag="xT")
                            nc.sync.dma_start(
                                out=t[:],
                                in_=contrib[k][h * HS:(h + 1) * HS, j * 128:(j + 1) * 128],
                                transpose=True)
                            xT.append(t)
                        for p in range(T // 2):
                            gp = h * (T // 2) + p
                            pj = psjp.tile([128, 128], dt.float32, tag="pj")
                            for k in range(K):
                                nc.tensor.matmul(pj[:], xT[k][:, p * 128:(p + 1) * 128],
                                                 w_t[:, k, :],
                                                 start=(k == 0), stop=(k == K - 1))
                            am = amax_all[:, j * T + gp: j * T + gp + 1]
                            nc.vector.tensor_reduce(am, pj[:], axis=mybir.AxisListType.XYZW,
                                                    op=mybir.AluOpType.max,
                                                    apply_absolute_value=True)
                            mul = ocp.tile([128, 1], dt.float32, tag="mul")
                            nc.vector.reciprocal(mul[:], am)
                            nc.vector.tensor_scalar(mul[:], mul[:], 127.0, None,
                                                    op0=mybir.AluOpType.mult)
                            of = ocp.tile([128, 128], dt.float32, tag="of")
                            nc.vector.tensor_scalar(of[:], pj[:], mul[:], None,
                                                    op0=mybir.AluOpType.mult)
                            o = ocp.tile([128, 128], dt.int8, tag="oc")
                            nc.vector.tensor_copy(o[:], of[:])
                            r0 = (2 * j) * SLICE + gp * 128
                            nc.sync.dma_start(out=outp[r0:r0 + 128, :], in_=o[:, 0:64])
                            r1 = (2 * j + 1) * SLICE + gp * 128
                            nc.sync.dma_start(out=outp[r1:r1 + 128, :], in_=o[:, 64:128])
                nc.sync.dma_start(
                    out=outp[B * SLICE:, :].bitcast(dt.float32).rearrange(
                        "(p a) f -> p (a f)", p=128),
                    in_=amax_all[:])

            for _rep in range(repeat):
                build_patterns()
                stage0()
                for k in range(1, K):
                    cheb_step(k)
                projection()

    nc.finalize()
    return nc


_NC_CACHE = {}


def get_nc(repeat=1):
    if repeat not in _NC_CACHE:
        _NC_CACHE[repeat] = build_nc(repeat)
    return _NC_CACHE[repeat]


def make_in_maps(x, edge_vals, W, edge_rows, edge_cols):
    g = build_meta(edge_rows, edge_cols, edge_vals)
    xs = build_x_slices(x)
    wb = build_w_blocks(W)
    blob = build_blob(g, wb)
    return [{"xin": xs[c], "blob": blob[c]} for c in range(NCORE)]


def run(x, edge_vals, W, edge_rows, edge_cols, repeat=1):
    in_maps = make_in_maps(x, edge_vals, W, edge_rows, edge_cols)
    nc = get_nc(repeat)
    res = run_bass_kernel_spmd(nc, in_maps, core_ids=list(range(NCORE)))
    out = np.empty((B, M, FOUT), np.float32)
    for c in range(NCORE):
        r0 = c * SLICE
        rows = min(M, r0 + SLICE) - r0
        raw = res.results[c]["outp"]
        # scl[sub, j*T+p] is the abs-max of pair j's 128 output values at
        # local row p*128+sub; dequant: i8 * amax / 127
        sc = raw[B * SLICE:].reshape(-1).view(np.float32) * (1.0 / 127.0)
        sc = sc.reshape(128, 4, T).transpose(1, 2, 0).reshape(4, 1, SLICE, 1)[:, :, :rows]
        blk = raw[:B * SLICE].reshape(4, 2, SLICE, FOUT)[:, :, :rows]
        np.multiply(blk, sc, out=out[:, r0:r0 + rows].reshape(4, 2, rows, FOUT))
    return out


def kernel(**inputs):
    return run(inputs["x"], inputs["edge_vals"], inputs["W"],
               inputs["edge_rows"], inputs["edge_cols"])
